# revision 1
# baseline (speedup 1.0000x reference)
"""Trainium2 Bass kernel for nn_Disp_61125974557155.

Computes: trilinear upsample of a cost volume [B,1,48,64,128] ->
[B,193,256,512] (align_corners=False, edge-replicated), softmin over
disparity, disparity regression -> [B,256,512].

Design (per core; 8 cores = 2 batches x 4 H'-quarters):
  - Host: edge-pad x (replicate), slice the core's H-halo shard, and stack a
    copy shifted by one h-row on partitions 50..99 (sharding/memory movement
    only, no arithmetic).
  - DVE: W-axis 4x lerp at low resolution -> xsw [100, 19, 4, 128] bf16.
  - PE: D-expansion with the H-axis lerp folded in (dup-shifted 100-row
    operand), all-bf16 (faster column stream + FWL weight loads + no
    FP32-HIGH mode switches).  d'=0 and d'=1 rows coincide (edge
    replication): 192 distinct rows = chunkA[128] + chunkB[64] (B
    zero-padded to M=128 so its PSUM bank is fully written).  Slots roll
    through 3-bank PSUM windows [128, 1536].
  - ACT: one exp per window (1536 cols) -> bf16 e-window; the dup'd d' row
    carries its multiplicity in the regression weights; pad rows weight 0.
  - PE: flip stat matmuls (e-slice stationary [K,128], rmat moving [K,2])
    accumulate (S0, S1) into one persistent pixel-major PSUM bank.
  - DVE: out = S1 * recip(S0); PE transposes q-planes via identity; DMA out.
"""

import numpy as np
from contextlib import ExitStack

import concourse.bass as bass
import concourse.bacc as bacc
import concourse.tile as tile
from concourse import mybir
from concourse.bass_utils import run_bass_kernel_spmd
from concourse.tile_rust import add_dep_helper

F32 = mybir.dt.float32
BF16 = mybir.dt.bfloat16

MAXDISP = 192
DP = MAXDISP + 1      # 193 disparities
ND = 192              # distinct d' rows (d'=0,1 coincide)
KD = 48               # low-res D
KP = KD + 2           # padded k' (edge-replicated)
NCORES = 8
WH = (0.625, 0.875, 0.125, 0.375)   # H lerp fracs per r = h' % 4
NROW = 19                            # h-rows in dup-packed shard
ROW_GROUPS = ((0, 1), (1, 1), (2, 2), (4, 4), (8, 4), (12, 4), (16, 3))
NTILES = 64                          # 4 r-phases x 16 t
NSLOTS = 2 * NTILES                  # A/B bank-slot per tile
NWIN = NSLOTS // 2                   # 2-slot windows (128 -> 64)


def _build_ad() -> np.ndarray:
    """A_D [192, 50]: head-dedup'd D-axis upsample matrix on padded k'.

    Row 0 covers d' in {0,1} (edge-replicated pair, canonicalized to weight
    1.0 on xp[1]); row i (i>=1) covers d' = i+1."""
    full = np.zeros((DP, KP), dtype=np.float64)
    for dp in range(DP):
        i = (dp + 0.5) * KD / DP - 0.5
        fl = int(np.floor(i))
        fr = i - fl
        full[dp, fl + 1] += 1.0 - fr
        full[dp, fl + 2] += fr
    for dp in (0, 1):
        assert abs(full[dp, 0] + full[dp, 1] - 1.0) < 1e-12 and full[dp, 2:].sum() == 0
        full[dp, 0], full[dp, 1] = 0.0, 1.0
    return full[1:]


def _build_consts():
    ad = _build_ad()                                       # [192, 50]
    # zero-pad chunkB to M=128 so its matmul writes the full PSUM bank
    adp = np.concatenate([ad, np.zeros((64, KP))], axis=0)  # [256, 50]
    amat = np.zeros((2 * KP, 4, 256), dtype=np.float64)
    for r in range(4):
        amat[:KP, r, :] = (1.0 - WH[r]) * adp.T
        amat[KP:, r, :] = WH[r] * adp.T
    # rmat [128, 4]: cols 0:2 = chunkA (S0w, S1w) rows 0:128 (row0 is the
    # {0,1} pair); cols 2:4 = chunkB weights rows 0:64 (d' 129..192, all
    # integers <= 256 so exactly representable in bf16).
    s0w = np.ones(ND)
    s1w = np.arange(1, ND + 1, dtype=np.float64)
    s0w[0], s1w[0] = 2.0, 1.0            # row0 = {0,1}: S0w 2, S1w 0+1
    rmat = np.zeros((128, 4), dtype=np.float64)
    rmat[:, 0] = s0w[0:128]
    rmat[:, 1] = s1w[0:128]
    rmat[0:64, 2] = s0w[128:192]
    rmat[0:64, 3] = s1w[128:192]
    rbf = rmat.astype(np.float32).astype(mybir.dt.np(BF16))
    assert np.array_equal(rbf.astype(np.float64), rmat)
    ident = np.eye(128, dtype=np.float32)
    return (
        np.ascontiguousarray(amat.reshape(2 * KP, 4 * 256), dtype=np.float32)
        .astype(mybir.dt.np(BF16)),
        rbf,
        ident,
    )


def _build_nc() -> bass.Bass:
    nc = bacc.Bacc()
    xsd = nc.declare_dram_parameter("xsd", [2 * KP, NROW * 130], F32, isOutput=False)
    amat = nc.declare_dram_parameter("amat", [2 * KP, 4 * 256], BF16, isOutput=False)
    rmat = nc.declare_dram_parameter("rmat", [128, 4], BF16, isOutput=False)
    ident = nc.declare_dram_parameter("ident", [128, 128], F32, isOutput=False)
    outp = nc.declare_dram_parameter("out", [64, 512], F32, isOutput=True)

    xsd_v = xsd.rearrange("p (h w) -> p h w", h=NROW)
    amat_v = amat.rearrange("p (r d) -> p r d", r=4)

    mult = mybir.AluOpType.mult
    add = mybir.AluOpType.add
    exp_fn = mybir.ActivationFunctionType.Exp

    # Chain PE matmuls in emission order so the scheduler keeps them
    # back-to-back (p-state / HAM ramp) and same-dtype runs contiguous.
    last_pe = [None]

    def pe_matmul(*args, **kwargs):
        ins = nc.tensor.matmul(*args, **kwargs)
        if last_pe[0] is not None:
            add_dep_helper(ins.ins, last_pe[0].ins, False,
                           "keep matmul bursts contiguous")
        last_pe[0] = ins
        return ins

    with ExitStack() as ctx:
        tc = ctx.enter_context(tile.TileContext(nc))
        singles = ctx.enter_context(tc.tile_pool(name="singles", bufs=1))
        tmp_pool = ctx.enter_context(tc.tile_pool(name="tmp", bufs=4))
        epool = ctx.enter_context(tc.tile_pool(name="epool", bufs=4))
        fin = ctx.enter_context(tc.tile_pool(name="fin", bufs=1))
        pvol = ctx.enter_context(tc.tile_pool(name="pvol", bufs=3, space="PSUM"))
        pstat = ctx.enter_context(tc.tile_pool(name="pstat", bufs=1, space="PSUM"))
        ptr = ctx.enter_context(tc.tile_pool(name="ptr", bufs=1, space="PSUM"))

        # ---- input loads ----
        # Group 0 goes through the gpsimd SWDGE path FIRST: it lands ~2us
        # earlier than via the sync HWDGE queue and gates the whole
        # lerp->matmul pipeline start.
        s_xsd = []
        for g, (g0, gn) in enumerate(ROW_GROUPS):
            t_x = singles.tile([2 * KP, gn, 130], F32, tag=f"xsd{g}")
            if g == 0:
                nc.gpsimd.dma_start(out=t_x, in_=xsd_v[:, g0 : g0 + gn, :])
            else:
                nc.sync.dma_start(out=t_x, in_=xsd_v[:, g0 : g0 + gn, :])
            s_xsd.append(t_x)
        s_am = {}
        for r in range(4):
            t_a = singles.tile([2 * KP, 128], BF16, tag=f"amA{r}")
            nc.gpsimd.dma_start(out=t_a, in_=amat_v[:, r, 0:128])
            s_am[("A", r)] = t_a
            t_b = singles.tile([2 * KP, 128], BF16, tag=f"amB{r}")
            nc.gpsimd.dma_start(out=t_b, in_=amat_v[:, r, 128:256])
            s_am[("B", r)] = t_b
        s_rm = singles.tile([128, 4], BF16, tag="rm")
        nc.gpsimd.dma_start(out=s_rm, in_=rmat[:, :])
        s_id = singles.tile([128, 128], F32, tag="id")
        nc.gpsimd.dma_start(out=s_id, in_=ident[:, :])

        # ---- W-axis 4x lerp at low res, rw-major planes (bf16 out) ----
        s_xsw = []
        for g, (g0, gn) in enumerate(ROW_GROUPS):
            t_w = singles.tile([2 * KP, gn, 4, 128], BF16, tag=f"xsw{g}")
            t_d = tmp_pool.tile([2 * KP, gn, 129], F32, tag="wld")
            nc.vector.tensor_sub(
                t_d, s_xsd[g][:, :, 0:129], s_xsd[g][:, :, 1:130]
            )
            for rw, (coef, dc, hc) in enumerate(
                ((0.375, 0, 1), (0.125, 0, 1), (0.875, 1, 2), (0.625, 1, 2))
            ):
                nc.vector.scalar_tensor_tensor(
                    out=t_w[:, :, rw, :],
                    in0=t_d[:, :, dc : dc + 128],
                    scalar=coef,
                    in1=s_xsd[g][:, :, hc : hc + 128],
                    op0=mult,
                    op1=add,
                )
            s_xsw.append(t_w)

        def xsw_row(l: int) -> bass.AP:
            for g, (g0, gn) in enumerate(ROW_GROUPS):
                if g0 <= l < g0 + gn:
                    return s_xsw[g][:, l - g0, :, :]
            raise IndexError(l)

        # ---- persistent pixel-major stats bank ----
        # ps[p, q, j, :] = (S0, S1) of output pixel (h'-row j, w' = 4*p + q)
        ps = pstat.tile([128, 512], F32, tag="ps")
        ps_v = ps.rearrange("p (q j s) -> p q j s", q=4, s=2)

        # ---- main loop over bank-slots in rolling 3-slot PSUM windows ----
        # Global slot s = 2*tile + (0:A, 1:B); tile ti = 16*r + t.
        def slot_info(s):
            ti, ab = divmod(s, 2)
            r, t = divmod(ti, 16)
            return r, t, ab

        ewins = {}

        def e_slice(s, c0, cn):
            w, k = divmod(s, 2)
            return ewins[w][:, 512 * k + c0 : 512 * k + c0 + cn]

        def stat_tile(ti, first):
            """8 flip matmuls for tile ti (A: 4 groups, B: 4 groups)."""
            r, t = divmod(ti, 16)
            j = 4 * t + r
            for ab in range(2):
                # kn=128 for BOTH chunks: B weight rows 64:128 are zeros and
                # the B e-rows 64:128 are exp(0)=1 pads, so the products
                # vanish exactly -- and the PE tile_size stays (128,128)
                # instead of alternating with (64,128) every 4 matmuls.
                kn = 128
                rcol = 2 * ab
                for q in range(4):
                    first_mm = first and ab == 0 and q == 0
                    pe_matmul(
                        ps_v[:, q, j, :],
                        e_slice(2 * ti + ab, 128 * q, 128)[0:kn, :],
                        s_rm[0:kn, rcol : rcol + 2],
                        start=first_mm,
                        stop=(ti == NTILES - 1 and ab == 1 and q == 3),
                        skip_group_check=True,
                    )

        def emit_slot_mm(pv, k, s):
            r, t, ab = slot_info(s)
            l = t if r < 2 else t + 1
            rhs = xsw_row(l).rearrange("p q s -> p (q s)")   # [100, 512]
            pe_matmul(pv[0:128, 512 * k : 512 * (k + 1)],
                      s_am[("A" if ab == 0 else "B", r)][:, :], rhs,
                      start=True, stop=True)

        stats_done = 0
        for w in range(NWIN):
            s0 = 2 * w
            pv = pvol.tile([128, 1024], F32, tag="pv")
            et = epool.tile([128, 1024], BF16, tag="e")
            ewins[w] = et
            for k in range(2):
                emit_slot_mm(pv, k, s0 + k)
            nc.scalar.activation(et, pv, exp_fn, scale=-1.0)
            # emit stats for tiles fully exp'd, staying one window behind
            ready = (s0 + 2) // 2 - 2
            while stats_done < min(max(ready, 0), NTILES):
                stat_tile(stats_done, stats_done == 0)
                stats_done += 1
        while stats_done < NTILES:
            stat_tile(stats_done, stats_done == 0)
            stats_done += 1

        # ---- finalize: out = S1 * recip(S0); transpose q-planes ----
        rec = fin.tile([128, 4, 64], F32, tag="rec")
        oo = fin.tile([128, 4, 64], F32, tag="oo")
        om = fin.tile([64, 128, 4], F32, tag="om")
        for q in range(4):
            nc.vector.reciprocal_approx_fast(rec[:, q, :], ps_v[:, q, :, 0])
            nc.vector.tensor_mul(oo[:, q, :], ps_v[:, q, :, 1], rec[:, q, :])
            tr = ptr.tile([64, 128], F32, tag="tr")
            nc.tensor.transpose(tr, oo[:, q, :], s_id)
            nc.vector.tensor_copy(om[:, :, q], tr)
        nc.sync.dma_start(out=outp[:, :], in_=om.rearrange("j s q -> j (s q)"))

    nc.compile()
    return nc


_CACHE: dict = {}


def _shard_inputs(x: np.ndarray):
    """Edge-pad and slice per-core shards (memory movement only)."""
    xpad = np.pad(x[:, 0], ((0, 0), (1, 1), (1, 3), (1, 1)), mode="edge")
    amat, rmat, ident = _build_consts()
    in_maps = []
    for c in range(NCORES):
        b, q = divmod(c, 4)
        xs = xpad[b][:, 16 * q : 16 * q + 20, :]          # [50, 20, 130]
        xsd = np.concatenate([xs[:, 0:19, :], xs[:, 1:20, :]], axis=0)
        xsd = np.ascontiguousarray(
            xsd.reshape(2 * KP, NROW * 130), dtype=np.float32
        )
        in_maps.append({"xsd": xsd, "amat": amat, "rmat": rmat, "ident": ident})
    return in_maps


def kernel(x: np.ndarray, _trace: bool = False, _tmpdir=None):
    x = np.asarray(x, dtype=np.float32)
    assert x.shape == (2, 1, 48, 64, 128), x.shape
    if "nc" not in _CACHE:
        _CACHE["nc"] = _build_nc()
    nc = _CACHE["nc"]
    in_maps = _shard_inputs(x)
    res = run_bass_kernel_spmd(
        nc, in_maps, list(range(NCORES)), trace=_trace, tmpdir=_tmpdir
    )
    out = np.zeros((2, 256, 512), dtype=np.float32)
    for c in range(NCORES):
        b, q = divmod(c, 4)
        out[b, 64 * q : 64 * (q + 1), :] = res.results[c]["out"]
    if _trace:
        return out, res
    return out



# revision 2
# speedup vs baseline: 1.2936x; 1.2936x over previous
"""Trainium2 Bass kernel for nn_Disp_61125974557155.

Computes: trilinear upsample of a cost volume [B,1,48,64,128] ->
[B,193,256,512] (align_corners=False, edge-replicated), softmin over
disparity, disparity regression -> [B,256,512].

Design (per core; 8 cores = 2 batches x 4 H'-quarters):
  - Host: edge-pad x (replicate), slice the core's H-halo shard, and stack a
    copy shifted by one h-row on partitions 50..99 (sharding/memory movement
    only, no arithmetic).
  - DVE: W-axis 4x lerp at low resolution -> xsw [100, 17, 4, 128] bf16.
  - PE: D-expansion with the H-axis lerp folded in (dup-shifted 100-row
    operand), all-bf16.  Tiles are paired (j1=2m, j2=2m+1 share the same
    xsw row): window = 3 PSUM banks [A1 | B1+B2 | A2] = [128, 1536].  The
    two 64-row B-chunks share the middle bank via a pair of accumulating
    matmuls whose stationaries are zero-padded into disjoint column halves
    -- no pad columns reach the ACT engine (25% less exp work than the
    4-bank layout).
  - ACT: one exp per window (1536 cols) -> bf16 e-window.  ACT is the
    pacing engine; everything else hides under it.
  - PE: flip stat matmuls (e-slice stationary [K,128], rmat moving)
    accumulate (S0, S1) into one persistent pixel-major PSUM bank; the
    packed B-bank yields both tiles' stats in one matmul (4 moving cols).
  - DVE: out = S1 * recip(S0); PE transposes q-planes via identity; DMA out.
"""

import numpy as np
from contextlib import ExitStack

import concourse.bass as bass
import concourse.bacc as bacc
import concourse.tile as tile
from concourse import mybir
from concourse.bass_utils import run_bass_kernel_spmd
from concourse.tile_rust import add_dep_helper

F32 = mybir.dt.float32
BF16 = mybir.dt.bfloat16

MAXDISP = 192
DP = MAXDISP + 1      # 193 disparities
ND = 192              # distinct d' rows (d'=0,1 coincide)
KD = 48               # low-res D
KP = KD + 2           # padded k' (edge-replicated)
NCORES = 8
WH = (0.625, 0.875, 0.125, 0.375)   # H lerp fracs per r = h' % 4
NROW = 17                            # h-rows in dup-packed shard (l = 0..16)
ROW_GROUPS = ((0, 1), (1, 1), (2, 2), (4, 4), (8, 4), (12, 4), (16, 1))
NPAIR = 32                           # tile pairs (j1=2m, j2=2m+1)


def _build_ad() -> np.ndarray:
    """A_D [192, 50]: head-dedup'd D-axis upsample matrix on padded k'.

    Row 0 covers d' in {0,1} (edge-replicated pair, canonicalized to weight
    1.0 on xp[1]); row i (i>=1) covers d' = i+1."""
    full = np.zeros((DP, KP), dtype=np.float64)
    for dp in range(DP):
        i = (dp + 0.5) * KD / DP - 0.5
        fl = int(np.floor(i))
        fr = i - fl
        full[dp, fl + 1] += 1.0 - fr
        full[dp, fl + 2] += fr
    for dp in (0, 1):
        assert abs(full[dp, 0] + full[dp, 1] - 1.0) < 1e-12 and full[dp, 2:].sum() == 0
        full[dp, 0], full[dp, 1] = 0.0, 1.0
    return full[1:]


def _build_consts():
    ad = _build_ad()                                       # [192, 50]
    ad_a = ad[0:128]                                       # d' {0,1},2..128
    ad_b = ad[128:192]                                     # d' 129..192
    # amat [100, 8, 128]: slabs 0..3 = A-chunk per r (H-lerp folded via the
    # dup'd 100-row operand); slabs 4..7 = B-chunk per r, zero-padded into
    # column half r%2 so the pair of B matmuls accumulates [B1; B2] into one
    # PSUM bank with full-partition writes (and FWL-friendly 128-col loads).
    amat = np.zeros((2 * KP, 8, 128), dtype=np.float64)
    for r in range(4):
        amat[:KP, r, :] = (1.0 - WH[r]) * ad_a.T
        amat[KP:, r, :] = WH[r] * ad_a.T
        h0 = 0 if r % 2 == 0 else 64
        amat[:KP, 4 + r, h0 : h0 + 64] = (1.0 - WH[r]) * ad_b.T
        amat[KP:, 4 + r, h0 : h0 + 64] = WH[r] * ad_b.T
    # rmat [128, 6]: cols 0:2 = A-chunk (S0w, S1w) for d-rows 0:128 (row0 is
    # the {0,1} pair: weights 2 and 0+1); cols 2:4 = B-stats of the pair's
    # FIRST tile (nonzero on partitions 0:64 = d' 129..192); cols 4:6 = the
    # SECOND tile's (partitions 64:128).  All entries are integers <= 256 so
    # exactly representable in bf16.
    s0w = np.ones(ND)
    s1w = np.arange(1, ND + 1, dtype=np.float64)
    s0w[0], s1w[0] = 2.0, 1.0            # row0 = {0,1}: S0w 2, S1w 0+1
    rmat = np.zeros((128, 6), dtype=np.float64)
    rmat[:, 0] = s0w[0:128]
    rmat[:, 1] = s1w[0:128]
    rmat[0:64, 2] = s0w[128:192]
    rmat[0:64, 3] = s1w[128:192]
    rmat[64:128, 4] = s0w[128:192]
    rmat[64:128, 5] = s1w[128:192]
    rbf = rmat.astype(np.float32).astype(mybir.dt.np(BF16))
    assert np.array_equal(rbf.astype(np.float64), rmat)
    ident = np.eye(128, dtype=np.float32)
    return (
        np.ascontiguousarray(amat.reshape(2 * KP, 8 * 128), dtype=np.float32)
        .astype(mybir.dt.np(BF16)),
        rbf,
        ident,
    )


def _build_nc() -> bass.Bass:
    nc = bacc.Bacc()
    xsd = nc.declare_dram_parameter("xsd", [2 * KP, NROW * 130], F32, isOutput=False)
    amat = nc.declare_dram_parameter("amat", [2 * KP, 8 * 128], BF16, isOutput=False)
    rmat = nc.declare_dram_parameter("rmat", [128, 6], BF16, isOutput=False)
    ident = nc.declare_dram_parameter("ident", [128, 128], F32, isOutput=False)
    outp = nc.declare_dram_parameter("out", [64, 512], F32, isOutput=True)

    xsd_v = xsd.rearrange("p (h w) -> p h w", h=NROW)
    amat_v = amat.rearrange("p (v d) -> p v d", v=8)

    mult = mybir.AluOpType.mult
    add = mybir.AluOpType.add
    exp_fn = mybir.ActivationFunctionType.Exp

    # Chain PE matmuls in emission order so the scheduler keeps them
    # back-to-back (p-state / HAM ramp) and same-dtype runs contiguous.
    last_pe = [None]

    def pe_matmul(*args, **kwargs):
        ins = nc.tensor.matmul(*args, **kwargs)
        if last_pe[0] is not None:
            add_dep_helper(ins.ins, last_pe[0].ins, False,
                           "keep matmul bursts contiguous")
        last_pe[0] = ins
        return ins

    with ExitStack() as ctx:
        tc = ctx.enter_context(tile.TileContext(nc))
        singles = ctx.enter_context(tc.tile_pool(name="singles", bufs=1))
        tmp_pool = ctx.enter_context(tc.tile_pool(name="tmp", bufs=4))
        epool = ctx.enter_context(tc.tile_pool(name="epool", bufs=4))
        fin = ctx.enter_context(tc.tile_pool(name="fin", bufs=1))
        pvol = ctx.enter_context(tc.tile_pool(name="pvol", bufs=2, space="PSUM"))
        pstat = ctx.enter_context(tc.tile_pool(name="pstat", bufs=1, space="PSUM"))
        ptr = ctx.enter_context(tc.tile_pool(name="ptr", bufs=1, space="PSUM"))

        # ---- input loads ----
        # Group 0 goes through the gpsimd SWDGE path FIRST: it lands ~2us
        # earlier than via the sync HWDGE queue and gates the whole
        # lerp->matmul pipeline start.
        s_xsd = []
        for g, (g0, gn) in enumerate(ROW_GROUPS):
            t_x = singles.tile([2 * KP, gn, 130], F32, tag=f"xsd{g}")
            if g == 0:
                nc.gpsimd.dma_start(out=t_x, in_=xsd_v[:, g0 : g0 + gn, :])
            else:
                nc.sync.dma_start(out=t_x, in_=xsd_v[:, g0 : g0 + gn, :])
            s_xsd.append(t_x)
        s_am = {}
        for r in range(4):
            t_a = singles.tile([2 * KP, 128], BF16, tag=f"amA{r}")
            nc.gpsimd.dma_start(out=t_a, in_=amat_v[:, r, :])
            s_am[("A", r)] = t_a
            t_b = singles.tile([2 * KP, 128], BF16, tag=f"amB{r}")
            nc.gpsimd.dma_start(out=t_b, in_=amat_v[:, 4 + r, :])
            s_am[("B", r)] = t_b
        s_rm = singles.tile([128, 6], BF16, tag="rm")
        nc.gpsimd.dma_start(out=s_rm, in_=rmat[:, :])
        s_id = singles.tile([128, 128], F32, tag="id")
        nc.gpsimd.dma_start(out=s_id, in_=ident[:, :])

        # ---- W-axis 4x lerp at low res, rw-major planes (bf16 out) ----
        s_xsw = []
        for g, (g0, gn) in enumerate(ROW_GROUPS):
            t_w = singles.tile([2 * KP, gn, 4, 128], BF16, tag=f"xsw{g}")
            t_d = tmp_pool.tile([2 * KP, gn, 129], F32, tag="wld")
            nc.vector.tensor_sub(
                t_d, s_xsd[g][:, :, 0:129], s_xsd[g][:, :, 1:130]
            )
            for rw, (coef, dc, hc) in enumerate(
                ((0.375, 0, 1), (0.125, 0, 1), (0.875, 1, 2), (0.625, 1, 2))
            ):
                nc.vector.scalar_tensor_tensor(
                    out=t_w[:, :, rw, :],
                    in0=t_d[:, :, dc : dc + 128],
                    scalar=coef,
                    in1=s_xsd[g][:, :, hc : hc + 128],
                    op0=mult,
                    op1=add,
                )
            s_xsw.append(t_w)

        def xsw_row(l: int) -> bass.AP:
            for g, (g0, gn) in enumerate(ROW_GROUPS):
                if g0 <= l < g0 + gn:
                    return s_xsw[g][:, l - g0, :, :]
            raise IndexError(l)

        # ---- persistent pixel-major stats bank ----
        # ps[p, q, j, :] = (S0, S1) of output pixel (h'-row j, w' = 4*p + q)
        ps = pstat.tile([128, 512], F32, tag="ps")
        ps_v = ps.rearrange("p (q j s) -> p q j s", q=4, s=2)
        ps_v4 = ps.rearrange("p (q m s4) -> p q m s4", q=4, s4=4)

        # ---- main loop: 32 tile pairs, one 3-bank window each ----
        # Pair m: j1 = 2m (r1 in {0,2}), j2 = 2m+1 (r2 = r1+1); both share
        # xsw row l = m//2 + m%2.
        ewins = {}

        def pair_info(m):
            t, odd = divmod(m, 2)
            r1 = 2 * odd
            return t + odd, r1, r1 + 1

        def emit_slots(m, pv):
            l, r1, r2 = pair_info(m)
            rhs = xsw_row(l).rearrange("p q s -> p (q s)")   # [100, 512]
            pe_matmul(pv[:, 0:512], s_am[("A", r1)][:, :], rhs,
                      start=True, stop=True)
            # B1 (cols 0:64 live) then B2 (cols 64:128 live) accumulate into
            # the shared middle bank; the zero column-halves keep every write
            # full-partition.
            pe_matmul(pv[:, 512:1024], s_am[("B", r1)][:, :], rhs,
                      start=True, stop=False, skip_group_check=True)
            pe_matmul(pv[:, 512:1024], s_am[("B", r2)][:, :], rhs,
                      start=False, stop=True, skip_group_check=True)
            pe_matmul(pv[:, 1024:1536], s_am[("A", r2)][:, :], rhs,
                      start=True, stop=True)

        def emit_stats(m, first):
            et = ewins[m]
            j1 = 2 * m
            for q in range(4):
                pe_matmul(
                    ps_v[:, q, j1, :],
                    et[:, 0 * 512 + 128 * q : 0 * 512 + 128 * q + 128],
                    s_rm[:, 0:2],
                    start=(first and q == 0), stop=False,
                    skip_group_check=True,
                )
                pe_matmul(
                    ps_v[:, q, j1 + 1, :],
                    et[:, 2 * 512 + 128 * q : 2 * 512 + 128 * q + 128],
                    s_rm[:, 0:2],
                    start=False, stop=False,
                    skip_group_check=True,
                )
                pe_matmul(
                    ps_v4[:, q, m, :],
                    et[:, 1 * 512 + 128 * q : 1 * 512 + 128 * q + 128],
                    s_rm[:, 2:6],
                    start=False,
                    stop=(m == NPAIR - 1 and q == 3),
                    skip_group_check=True,
                )

        stats_done = 0
        for m in range(NPAIR):
            pv = pvol.tile([128, 1536], F32, tag="pv")
            et = epool.tile([128, 1536], BF16, tag="e")
            ewins[m] = et
            emit_slots(m, pv)
            nc.scalar.activation(et, pv, exp_fn, scale=-1.0)
            # emit stats lagging two windows: exp(m-2) is complete by the
            # time slots(m) could start (its pvol buffer was recycled), so
            # these never stall the PE queue.
            if m >= 2:
                emit_stats(m - 2, m == 2)
                stats_done = m - 1
        while stats_done < NPAIR:
            emit_stats(stats_done, False)
            stats_done += 1

        # ---- finalize: out = S1 * recip(S0); transpose q-planes ----
        rec = fin.tile([128, 4, 64], F32, tag="rec")
        oo = fin.tile([128, 4, 64], F32, tag="oo")
        om = fin.tile([64, 128, 4], F32, tag="om")
        for q in range(4):
            nc.vector.reciprocal_approx_fast(rec[:, q, :], ps_v[:, q, :, 0])
            nc.vector.tensor_mul(oo[:, q, :], ps_v[:, q, :, 1], rec[:, q, :])
            tr = ptr.tile([64, 128], F32, tag="tr")
            nc.tensor.transpose(tr, oo[:, q, :], s_id)
            nc.vector.tensor_copy(om[:, :, q], tr)
        nc.sync.dma_start(out=outp[:, :], in_=om.rearrange("j s q -> j (s q)"))

    nc.compile()
    return nc


_CACHE: dict = {}


def _shard_inputs(x: np.ndarray):
    """Edge-pad and slice per-core shards (memory movement only)."""
    xpad = np.pad(x[:, 0], ((0, 0), (1, 1), (1, 3), (1, 1)), mode="edge")
    amat, rmat, ident = _build_consts()
    in_maps = []
    for c in range(NCORES):
        b, q = divmod(c, 4)
        xs = xpad[b][:, 16 * q : 16 * q + 18, :]          # [50, 18, 130]
        xsd = np.concatenate([xs[:, 0:17, :], xs[:, 1:18, :]], axis=0)
        xsd = np.ascontiguousarray(
            xsd.reshape(2 * KP, NROW * 130), dtype=np.float32
        )
        in_maps.append({"xsd": xsd, "amat": amat, "rmat": rmat, "ident": ident})
    return in_maps


def kernel(x: np.ndarray, _trace: bool = False, _tmpdir=None):
    x = np.asarray(x, dtype=np.float32)
    assert x.shape == (2, 1, 48, 64, 128), x.shape
    if "nc" not in _CACHE:
        _CACHE["nc"] = _build_nc()
    nc = _CACHE["nc"]
    in_maps = _shard_inputs(x)
    res = run_bass_kernel_spmd(
        nc, in_maps, list(range(NCORES)), trace=_trace, tmpdir=_tmpdir
    )
    out = np.zeros((2, 256, 512), dtype=np.float32)
    for c in range(NCORES):
        b, q = divmod(c, 4)
        out[b, 64 * q : 64 * (q + 1), :] = res.results[c]["out"]
    if _trace:
        return out, res
    return out
